# revision 19
# baseline (speedup 1.0000x reference)
"""Trainium2 Bass kernel for per-head attention (fp16 v7, fully pipelined).

Problem shapes: x [4, 1024, 12, 768]; per-head weights W_Q/K/V [12, 768, 64],
W_O [12, 64, 768]; the output projection keeps the head axis, so each of the
48 (batch, head) pairs is fully independent. Sharding: 6 pairs per core
across 8 NeuronCores (SPMD), grouped so each core sees only 2 distinct heads
(one head x 4 batches + one half-head x 2 batches) -> 2 weight DMAs per core.

All DMA-side tensors are fp16; PSUM stays fp32.

The kernel is organized as a software pipeline over the 6 (batch, head)
pairs.  Each section = one pair's attention phase (scores -> exp -> z, per
256-column q-chunk, causally chunked over 128-row key blocks, paced by the
serial Exp chain on ACT), with the NEXT pair's Q/K/V projections and the
PREVIOUS pair's output-projection tiles interleaved as PE fill work at ten
evenly spaced slots.  This keeps the tensor engine dense (so the HAM clock
gate stays at 8/8 = 2.4 GHz) and lets every psum stage double-buffer inside
the 8-bank budget:
    scores/qkv ring (1-bank tiles)  3 banks
    z accumulator  (1-bank tile)    1 bank
    proj tiles     (2-bank tiles)   4 banks
Tricks: packed [Wk|Wq] stationary; one biased full-width copy evacuates
[k+bK; q+bQ] with a single partition-shift DMA for q; v computed in [s, d]
layout with xT chunks stationary (no transposes), all 8 v-tiles built by one
strided copy with interleaved ones-columns (softmax denominator); the causal
mask is applied by an accumulating -1000*lower-tri matmul on diagonal blocks
(exp underflow zeroes masked lanes); one Exp covers each pair of key blocks;
Wo_aug's indicator column emits the denominator, and a single fused
multiply-by-reciprocal op per projection tile (alternating DVE/ACT)
evacuates the result.
"""

import numpy as np

import concourse.bacc as bacc
import concourse.mybir as mybir
from concourse.bass_utils import run_bass_kernel_spmd
from concourse.tile import TileContext

F16 = mybir.dt.float16
F32 = mybir.dt.float32

B, S, H, DM, DH = 4, 1024, 12, 768, 64
N_CORES = 8
PAIRS_PER_CORE = (B * H) // N_CORES  # 6
MC = DM // 128  # m-chunks
ST = S // 128   # s-tiles
NJ = S // 256   # q-chunks (256 wide)
NMASK = -1000.0  # pre-scale mask addend; exp(0.125 * -1000) == 0 exactly

# packed per-head weight blob (fp16 columns):
# [ wqk (MC*128, per-chunk [Wk|Wq]) | wv (MC*64, moving-layout W_V chunks) |
#   wo_aug (rows 0:65, 770 cols)    | bkq (f32, 2 cols: rows 0:64 = b_K,
#   rows 64:128 = b_Q) ]
WQK0, WV0 = 0, MC * 128
WO0 = WV0 + MC * DH
BKQ0 = WO0 + DM + 2
WBL = BKQ0 + 2

# (j, a) block-pair schedule: chunk j covers q-cols [256j, 256j+256), key
# block-pairs a = 0..j (blocks 2a, 2a+1); the a == j pair is diagonal.
SCHED = [(j, a) for j in range(NJ) for a in range(j + 1)]
# fill-slot plan: which of the 10 per-section slots run a next-pair QKV
# closure (the rest run previous-pair projection tiles)
QKV_SLOTS = (0, 2, 5)
ACT_STS = (1, 3, 5)  # proj evacuations on ACT; the rest on DVE


def _build_kernel(n_pairs=PAIRS_PER_CORE):
    nc = bacc.Bacc()

    xT = nc.declare_dram_parameter("xT", [n_pairs, DM, S], F16, isOutput=False)
    wb = nc.declare_dram_parameter("wb", [2, 128, WBL], F16, isOutput=False)
    # cmask cols 0:128 = -1000*I (stationary), 128:256 = strict-lower-tri
    cmask = nc.declare_dram_parameter("cmask", [128, 256], F16, isOutput=False)
    out = nc.declare_dram_parameter("out", [n_pairs, S, DM], F16, isOutput=True)

    with TileContext(nc) as tc:
        with (
            tc.tile_pool(name="const", bufs=1) as pconst,
            tc.tile_pool(name="xt", bufs=2) as px,
            tc.tile_pool(name="w", bufs=2) as pw,
            tc.tile_pool(name="qkv", bufs=2) as pqkv,
            tc.tile_pool(name="vaug", bufs=2) as pva,
            tc.tile_pool(name="exp", bufs=4) as pexp,
            tc.tile_pool(name="z", bufs=2) as pz,
            tc.tile_pool(name="rc", bufs=2) as prc,
            tc.tile_pool(name="outb", bufs=3) as pout,
            tc.tile_pool(name="ps_r", bufs=3, space="PSUM") as ppr,
            tc.tile_pool(name="ps_z", bufs=1, space="PSUM") as ppz,
            tc.tile_pool(name="ps_p", bufs=2, space="PSUM") as ppp,
        ):
            cm = pconst.tile([128, 256], F16, name="cm")
            nc.sync.dma_start(out=cm[:], in_=cmask[:])
            negI = cm[:, 0:128]
            ltm = cm[:, 128:256]

            # PE warmup while the first x DMA is in flight (flips the HAM
            # clock gate to 8/8 before real work), plus a dummy Exp so the
            # ACT table set loads here instead of stalling the first pair.
            wscr = pconst.tile([128, 512], F16, name="wscr")
            escr = pconst.tile([1, 8], F16, name="escr")
            nc.vector.memset(wscr[:], 0.0)
            nc.scalar.activation(escr[:], wscr[0:1, 0:8],
                                 mybir.ActivationFunctionType.Exp,
                                 bias=0.0, scale=0.125)
            for wi in range(10):
                ps_w = ppr.tile([128, 512], F32, name="ps_w", tag="ps_r")
                nc.tensor.matmul(ps_w[:], wscr[:, 0:128], wscr[:],
                                 start=True, stop=True)

            # per-group weight views
            wviews = {}

            def load_group(g):
                wb_t = pw.tile([128, WBL], F16, name="wb_t", tag="wb")
                nc.sync.dma_start(out=wb_t[:], in_=wb[g])
                wviews[g] = dict(
                    wqk=wb_t[:, WQK0:WV0].rearrange("p (c d) -> p c d",
                                                    d=128),
                    wv=wb_t[:, WV0:WO0].rearrange("p (c d) -> p c d", d=DH),
                    wo=wb_t[0:DH + 1, WO0:WO0 + DM + 2],
                    bkq=wb_t[:, BKQ0:BKQ0 + 2].bitcast(F32),
                )

            grp = [0] * 4 + [1] * 2
            grp = grp[:n_pairs]
            state = {}  # p -> dict(xta, kq, qlo, va)

            def load_x(p, fine=False):
                xta = px.tile([128, MC, S], F16, name="xta", tag="xta")
                xTv = xT[p].rearrange("(c p) s -> p c s", p=128)
                if fine:
                    for mc in range(MC):
                        nc.sync.dma_start(out=xta[:, mc, :],
                                          in_=xTv[:, mc, :])
                else:
                    nc.sync.dma_start(out=xta[:], in_=xTv)
                state[p] = {"xta": xta}

            def mk_qk_chunk(p, sc):
                def run():
                    st_ = state[p]
                    wv_ = wviews[grp[p]]
                    if sc == 0:
                        st_["kq"] = pqkv.tile([128, S], F16, name="kq",
                                              tag="kq")
                        st_["qlo"] = pqkv.tile([DH, S], F16, name="qlo",
                                               tag="qlo")
                    kq, qlo = st_["kq"], st_["qlo"]
                    ps = ppr.tile([128, 512], F32, name="ps_qk", tag="ps_r")
                    for mc in range(MC):
                        nc.tensor.matmul(
                            ps[:], wv_["wqk"][:, mc, :],
                            st_["xta"][:, mc, sc * 512:(sc + 1) * 512],
                            start=(mc == 0), stop=(mc == MC - 1))
                    cols = slice(sc * 512, (sc + 1) * 512)
                    nc.vector.tensor_scalar(
                        kq[:, cols], ps[:], wv_["bkq"][:], None,
                        op0=mybir.AluOpType.add)
                    nc.sync.dma_start(
                        out=qlo[:, cols], in_=kq[DH:128, cols])
                return run

            def mk_v(p):
                def run():
                    st_ = state[p]
                    wv_ = wviews[grp[p]]
                    ps_v = ppr.tile([128, 512], F32, name="ps_v", tag="ps_r")
                    for st in range(ST):
                        for mc in range(MC):
                            nc.tensor.matmul(
                                ps_v[:, st * DH:(st + 1) * DH],
                                st_["xta"][:, mc, st * 128:(st + 1) * 128],
                                wv_["wv"][:, mc, :],
                                start=(mc == 0), stop=(mc == MC - 1))
                    va = pva.tile([128, ST * (DH + 1)], F16, name="va",
                                  tag="va")
                    nc.vector.memset(va[:], 1.0)
                    nc.vector.tensor_copy(
                        va[:].rearrange("p (s d) -> p s d", d=DH + 1)
                        [:, :, 0:DH],
                        ps_v[:].rearrange("p (s d) -> p s d", d=DH))
                    st_["va"] = va
                return run

            def qkv_fills(p):
                return [mk_qk_chunk(p, 0), mk_v(p), mk_qk_chunk(p, 1)]

            def mk_proj(p, st, z_lo, z_hi, obh_box):
                wo_t = wviews[grp[p]]["wo"]
                last = p == n_pairs - 1

                def run():
                    zh = z_lo if st < 4 else z_hi
                    zsl = zh[:, (st % 4) * 128:(st % 4 + 1) * 128]
                    gg = st % 4
                    if gg == 0:
                        obh_box[0] = pout.tile([128, 4, DM], F16,
                                               name="obh", tag="obh")
                    obh = obh_box[0]
                    pp = ppp.tile([128, 1024], F32, name="ps_p", tag="ps_p")
                    nc.tensor.matmul(pp[:, 512:898], zsl,
                                     wo_t[:, 384:DM + 2],
                                     start=True, stop=True)
                    rc = prc.tile([128, 1], F32, name=f"rc{st}",
                                  tag=f"rc{st}")
                    nc.vector.reciprocal(rc[:], pp[:, 896:897])
                    nc.tensor.matmul(pp[:, 0:384], zsl, wo_t[:, 0:384],
                                     start=True, stop=True)
                    in_ap = pp[:].rearrange(
                        "p (b c) -> p b c", c=512)[:, :, 0:384]
                    out_ap = obh[:, gg, :].rearrange(
                        "p (b c) -> p b c", c=384)
                    if st in ACT_STS:
                        nc.scalar.mul(out_ap, in_ap, rc[:])
                    else:
                        nc.vector.tensor_scalar(
                            out_ap, in_ap, rc[:], None,
                            op0=mybir.AluOpType.mult)
                    if last and gg % 2 == 1:
                        nc.gpsimd.dma_start(
                            out=out[p, (st - 1) * 128:(st + 1) * 128, :]
                            .rearrange("(g sp) m -> sp g m", sp=128),
                            in_=obh[:, gg - 1:gg + 1, :])
                    elif not last and gg == 3:
                        nc.gpsimd.dma_start(
                            out=out[p, (st - 3) * 128:(st + 1) * 128, :]
                            .rearrange("(g sp) m -> sp g m", sp=128),
                            in_=obh[:])
                return run

            def attention(p, z_lo, z_hi, fq_qkv, fq_proj, fq_late):
                st_ = state[p]
                kq, qlo = st_["kq"], st_["qlo"]
                ztile = [None]
                pend = {}

                def emit_pair(k):
                    j, a = SCHED[k]
                    LB = 256 if a < j else 128
                    c0 = 256 * j
                    c0B = c0 if a < j else c0 + 128
                    diag = a == j
                    # one psum bank per block pair: only the FIRST matmul may
                    # use start=True (it clears has_written for the whole
                    # bank); the rest overwrite/accumulate per element.
                    T = ppr.tile([128, 512], F32, name="ps_s", tag="ps_r")
                    nc.tensor.matmul(
                        T[:, 0:256], kq[0:DH, 2 * a * 128:2 * a * 128 + 128],
                        qlo[:, c0:c0 + 256], start=True, stop=False)
                    nc.tensor.matmul(
                        T[:, 256:256 + LB],
                        kq[0:DH, (2 * a + 1) * 128:(2 * a + 2) * 128],
                        qlo[:, c0B:c0B + LB], start=False, stop=not diag)
                    if diag:
                        nc.tensor.matmul(T[:, 0:128], negI, ltm,
                                         start=False, stop=False)
                        nc.tensor.matmul(T[:, 256:384], negI, ltm,
                                         start=False, stop=True)
                    pend[k] = (T, LB)

                emit_pair(0)
                emit_pair(1)
                for k, (j, a) in enumerate(SCHED):
                    if a == 0 and j % 2 == 0:
                        ztile[0] = ppz.tile([DH + 1, 512], F32, name="ps_zb",
                                            tag="ps_z")
                    zt = ztile[0]
                    zoff = (j % 2) * 256
                    T, LB = pend.pop(k)
                    ex = pexp.tile([128, 512], F16, name="ex", tag="ex")
                    nc.scalar.activation(
                        ex[:, 0:256 + LB], T[:, 0:256 + LB],
                        mybir.ActivationFunctionType.Exp,
                        bias=0.0, scale=0.125)
                    if k + 2 < len(SCHED):
                        emit_pair(k + 2)
                    nc.tensor.matmul(
                        zt[:, zoff:zoff + 256],
                        st_["va"][:, 2 * a * (DH + 1):
                                  (2 * a + 1) * (DH + 1)],
                        ex[:, 0:256], start=(a == 0), stop=False)
                    nc.tensor.matmul(
                        zt[:, zoff + 256 - LB:zoff + 256],
                        st_["va"][:, (2 * a + 1) * (DH + 1):
                                  (2 * a + 2) * (DH + 1)],
                        ex[:, 256:256 + LB], start=False, stop=(a == j))
                    # fill slot: next-pair QKV at its slots, else previous-
                    # pair proj tiles, else (last pair, z_lo ready) own proj
                    # tiles, else a dummy warm matmul to keep the HAM gate
                    # at 8/8 through sparse sections.
                    for _rep in range(2 if k == len(SCHED) - 1 else 1):
                        if k in QKV_SLOTS and fq_qkv:
                            fq_qkv.pop(0)()
                        elif fq_proj:
                            fq_proj.pop(0)()
                        elif k >= 3 and fq_late:
                            fq_late.pop(0)()
                        else:
                            ps_w = ppp.tile([128, 1024], F32, name="ps_w2",
                                            tag="ps_p")
                            nc.tensor.matmul(ps_w[:, 0:512], wscr[:, 0:128],
                                             wscr[:], start=True, stop=True)
                    if a == j and j % 2 == 1:
                        nc.vector.tensor_copy(
                            (z_lo if j == 1 else z_hi)[:], zt[:])

            # ---- prologue ----
            load_group(0)
            load_x(0, fine=True)
            for f in qkv_fills(0):
                f()
            load_x(1)

            fq_proj = []
            for p in range(n_pairs):
                if p + 2 < n_pairs:
                    load_x(p + 2)
                    if grp[p + 2] == 1 and 1 not in wviews:
                        load_group(1)
                fq_qkv = qkv_fills(p + 1) if p + 1 < n_pairs else []
                z_lo = pz.tile([DH + 1, 512], F16, name="z_lo", tag="z_lo")
                z_hi = pz.tile([DH + 1, 512], F16, name="z_hi", tag="z_hi")
                obh_box = [None]
                last = p == n_pairs - 1
                fq_late = ([mk_proj(p, st, z_lo, z_hi, obh_box)
                            for st in range(4)] if last else [])
                attention(p, z_lo, z_hi, fq_qkv, fq_proj, fq_late)
                assert not fq_qkv and not fq_proj
                rest = range(4, ST) if last else range(ST)
                fq_proj = fq_late + [mk_proj(p, st, z_lo, z_hi, obh_box)
                                     for st in rest]
            for f in fq_proj:
                f()

    nc.finalize()
    return nc


_NC_CACHE = {}


def _get_nc():
    if "nc" not in _NC_CACHE:
        _NC_CACHE["nc"] = _build_kernel()
    return _NC_CACHE["nc"]


def _core_pairs(c):
    """6 (batch, head) pairs for core c: head c x batches 0..3, plus half of
    head 8 + c//2 (2 batches)."""
    pairs = [(b, c) for b in range(B)]
    h2 = 8 + c // 2
    b0 = (c % 2) * 2
    pairs += [(b0, h2), (b0 + 1, h2)]
    return pairs


def _head_blob(W_Q, b_Q, W_K, b_K, W_V, b_V, W_O, b_O, h):
    wbh = np.zeros((128, WBL), np.float16)
    wqk = wbh[:, WQK0:WV0].reshape(128, MC, 128)
    wqk[:, :, 0:DH] = W_K[h].reshape(MC, 128, DH).transpose(1, 0, 2)
    wqk[:, :, DH:128] = W_Q[h].reshape(MC, 128, DH).transpose(1, 0, 2)
    wbh[:, WV0:WO0].reshape(128, MC, DH)[:] = \
        W_V[h].reshape(MC, 128, DH).transpose(1, 0, 2)
    wbh[0:DH, WO0:WO0 + DM] = W_O[h]
    wbh[DH, WO0:WO0 + DM] = b_V[h] @ W_O[h] + b_O / H
    wbh[DH, WO0 + DM] = 1.0
    bkq = np.concatenate([np.asarray(b_K[h], np.float32),
                          np.asarray(b_Q[h], np.float32)])
    wbh[:, BKQ0:BKQ0 + 2] = \
        np.ascontiguousarray(bkq).view(np.float16).reshape(128, 2)
    return wbh


def _make_core_inputs(x, W_Q, b_Q, W_K, b_K, W_V, b_V, W_O, b_O, c):
    pairs = _core_pairs(c)
    m = {
        "xT": np.empty((PAIRS_PER_CORE, DM, S), np.float16),
        "wb": np.empty((2, 128, WBL), np.float16),
    }
    for idx, (b, h) in enumerate(pairs):
        m["xT"][idx] = x[b, :, h, :].T
    args = (W_Q, b_Q, W_K, b_K, W_V, b_V, W_O, b_O)
    m["wb"][0] = _head_blob(*args, pairs[0][1])
    m["wb"][1] = _head_blob(*args, pairs[4][1])
    cm = np.zeros((128, 256), np.float16)
    cm[:, 0:128] = NMASK * np.eye(128, dtype=np.float16)
    ql = np.arange(128)
    cm[:, 128:256] = (ql[None, :] < ql[:, None]).astype(np.float16)
    m["cmask"] = cm
    return m


def kernel(normalized_resid_pre, W_Q, b_Q, W_K, b_K, W_V, b_V, W_O, b_O):
    x = np.ascontiguousarray(np.asarray(normalized_resid_pre, dtype=np.float32))
    args = tuple(np.asarray(a, dtype=np.float32)
                 for a in (W_Q, b_Q, W_K, b_K, W_V, b_V, W_O, b_O))

    nc = _get_nc()
    in_maps = [_make_core_inputs(x, *args, c) for c in range(N_CORES)]
    res = run_bass_kernel_spmd(nc, in_maps, list(range(N_CORES)))

    got = np.empty((B, S, H, DM), np.float32)
    for c in range(N_CORES):
        ro = np.asarray(res.results[c]["out"], np.float32)
        for idx, (b, h) in enumerate(_core_pairs(c)):
            got[b, :, h, :] = ro[idx]
    return got


# revision 22
# speedup vs baseline: 1.0503x; 1.0503x over previous
"""Trainium2 Bass kernel for per-head attention (fp16 v7, fully pipelined).

Problem shapes: x [4, 1024, 12, 768]; per-head weights W_Q/K/V [12, 768, 64],
W_O [12, 64, 768]; the output projection keeps the head axis, so each of the
48 (batch, head) pairs is fully independent. Sharding: 6 pairs per core
across 8 NeuronCores (SPMD), grouped so each core sees only 2 distinct heads
(one head x 4 batches + one half-head x 2 batches) -> 2 weight DMAs per core.

All DMA-side tensors are fp16; PSUM stays fp32.

The kernel is organized as a software pipeline over the 6 (batch, head)
pairs.  Each section = one pair's attention phase (scores -> exp -> z, per
256-column q-chunk, causally chunked over 128-row key blocks, paced by the
serial Exp chain on ACT), with the NEXT pair's Q/K/V projections and the
PREVIOUS pair's output-projection tiles interleaved as PE fill work at ten
evenly spaced slots.  This keeps the tensor engine dense (so the HAM clock
gate stays at 8/8 = 2.4 GHz) and lets every psum stage double-buffer inside
the 8-bank budget:
    scores/qkv ring (1-bank tiles)  3 banks
    z accumulator  (1-bank tile)    1 bank
    proj tiles     (2-bank tiles)   4 banks
Tricks: packed [Wk|Wq] stationary; one biased full-width copy evacuates
[k+bK; q+bQ] with a single partition-shift DMA for q; v computed in [s, d]
layout with xT chunks stationary (no transposes), all 8 v-tiles built by one
strided copy with interleaved ones-columns (softmax denominator); the causal
mask is applied by an accumulating -1000*lower-tri matmul on diagonal blocks
(exp underflow zeroes masked lanes); one Exp covers each pair of key blocks;
Wo_aug's indicator column emits the denominator, and a single fused
multiply-by-reciprocal op per projection tile (alternating DVE/ACT)
evacuates the result.
"""

import numpy as np

import concourse.bacc as bacc
import concourse.mybir as mybir
from concourse.bass_utils import run_bass_kernel_spmd
from concourse.tile import TileContext

F16 = mybir.dt.float16
F32 = mybir.dt.float32

B, S, H, DM, DH = 4, 1024, 12, 768, 64
N_CORES = 8
PAIRS_PER_CORE = (B * H) // N_CORES  # 6
MC = DM // 128  # m-chunks
ST = S // 128   # s-tiles
NJ = S // 256   # q-chunks (256 wide)
NMASK = -1000.0  # pre-scale mask addend; exp(0.125 * -1000) == 0 exactly

# packed per-head weight blob (fp16 columns):
# [ wqk (MC*128, per-chunk [Wk|Wq]) | wv (MC*64, moving-layout W_V chunks) |
#   wo_aug (rows 0:65, 770 cols)    | bkq (f32, 2 cols: rows 0:64 = b_K,
#   rows 64:128 = b_Q) ]
WQK0, WV0 = 0, MC * 128
WO0 = WV0 + MC * DH
BKQ0 = WO0 + DM + 2
WBL = BKQ0 + 2

# (j, a) block-pair schedule: chunk j covers q-cols [256j, 256j+256), key
# block-pairs a = 0..j (blocks 2a, 2a+1); the a == j pair is diagonal.
SCHED = [(j, a) for j in range(NJ) for a in range(j + 1)]
# fill-slot plan: which of the 10 per-section slots run a next-pair QKV
# closure (the rest run previous-pair projection tiles)
QKV_SLOTS = (0, 2, 5)
ACT_STS = (1, 3, 5)  # proj evacuations on ACT; the rest on DVE


def _build_kernel(n_pairs=PAIRS_PER_CORE):
    nc = bacc.Bacc()

    xT = nc.declare_dram_parameter("xT", [n_pairs, DM, S], F16, isOutput=False)
    wb = nc.declare_dram_parameter("wb", [2, 128, WBL], F16, isOutput=False)
    # cmask cols 0:128 = -1000*I (stationary), 128:256 = strict-lower-tri
    cmask = nc.declare_dram_parameter("cmask", [128, 256], F16, isOutput=False)
    out = nc.declare_dram_parameter("out", [n_pairs, S, DM], F16, isOutput=True)

    with TileContext(nc) as tc:
        with (
            tc.tile_pool(name="const", bufs=1) as pconst,
            tc.tile_pool(name="xt", bufs=2) as px,
            tc.tile_pool(name="w", bufs=2) as pw,
            tc.tile_pool(name="qkv", bufs=2) as pqkv,
            tc.tile_pool(name="vaug", bufs=2) as pva,
            tc.tile_pool(name="exp", bufs=4) as pexp,
            tc.tile_pool(name="z", bufs=2) as pz,
            tc.tile_pool(name="rc", bufs=2) as prc,
            tc.tile_pool(name="outb", bufs=3) as pout,
            tc.tile_pool(name="ps_r", bufs=3, space="PSUM") as ppr,
            tc.tile_pool(name="ps_z", bufs=1, space="PSUM") as ppz,
            tc.tile_pool(name="ps_p", bufs=2, space="PSUM") as ppp,
        ):
            cm = pconst.tile([128, 256], F16, name="cm")
            nc.sync.dma_start(out=cm[:], in_=cmask[:])
            negI = cm[:, 0:128]
            ltm = cm[:, 128:256]

            # PE warmup while the first x DMA is in flight (flips the HAM
            # clock gate to 8/8 before real work), plus a dummy Exp so the
            # ACT table set loads here instead of stalling the first pair.
            wscr = pconst.tile([128, 512], F16, name="wscr")
            escr = pconst.tile([1, 8], F16, name="escr")
            nc.vector.memset(wscr[:], 0.0)
            nc.scalar.activation(escr[:], wscr[0:1, 0:8],
                                 mybir.ActivationFunctionType.Exp,
                                 bias=0.0, scale=0.125)
            for wi in range(10):
                ps_w = ppr.tile([128, 512], F32, name="ps_w", tag="ps_r")
                nc.tensor.matmul(ps_w[:], wscr[:, 0:128], wscr[:],
                                 start=True, stop=True)

            # per-group weight views
            wviews = {}

            def load_group(g):
                wb_t = pw.tile([128, WBL], F16, name="wb_t", tag="wb")
                nc.sync.dma_start(out=wb_t[:], in_=wb[g])
                wviews[g] = dict(
                    wqk=wb_t[:, WQK0:WV0].rearrange("p (c d) -> p c d",
                                                    d=128),
                    wv=wb_t[:, WV0:WO0].rearrange("p (c d) -> p c d", d=DH),
                    wo=wb_t[0:DH + 1, WO0:WO0 + DM + 2],
                    bkq=wb_t[:, BKQ0:BKQ0 + 2].bitcast(F32),
                )

            grp = [0] * 4 + [1] * 2
            grp = grp[:n_pairs]
            state = {}  # p -> dict(xta, kq, qlo, va)

            def load_x(p, fine=False):
                xta = px.tile([128, MC, S], F16, name="xta", tag="xta")
                xTv = xT[p].rearrange("(c p) s -> p c s", p=128)
                if fine:
                    for mc in range(MC):
                        nc.sync.dma_start(out=xta[:, mc, :],
                                          in_=xTv[:, mc, :])
                else:
                    nc.sync.dma_start(out=xta[:], in_=xTv)
                state[p] = {"xta": xta}

            def mk_qk_chunk(p, sc):
                def run():
                    st_ = state[p]
                    wv_ = wviews[grp[p]]
                    # separate tiles per 512-chunk so early score matmuls
                    # don't (tile-granularly) wait on the chunk-1 copies
                    kq = pqkv.tile([128, 512], F16, name=f"kq{sc}",
                                   tag=f"kq{sc}")
                    qlo = pqkv.tile([DH, 512], F16, name=f"qlo{sc}",
                                    tag=f"qlo{sc}")
                    st_[f"kq{sc}"], st_[f"qlo{sc}"] = kq, qlo
                    ps = ppr.tile([128, 512], F32, name="ps_qk", tag="ps_r")
                    for mc in range(MC):
                        nc.tensor.matmul(
                            ps[:], wv_["wqk"][:, mc, :],
                            st_["xta"][:, mc, sc * 512:(sc + 1) * 512],
                            start=(mc == 0), stop=(mc == MC - 1))
                    nc.vector.tensor_scalar(
                        kq[:], ps[:], wv_["bkq"][:], None,
                        op0=mybir.AluOpType.add)
                    nc.sync.dma_start(out=qlo[:], in_=kq[DH:128, :])
                return run

            def mk_v(p):
                def run():
                    st_ = state[p]
                    wv_ = wviews[grp[p]]
                    ps_v = ppr.tile([128, 512], F32, name="ps_v", tag="ps_r")
                    for st in range(ST):
                        for mc in range(MC):
                            nc.tensor.matmul(
                                ps_v[:, st * DH:(st + 1) * DH],
                                st_["xta"][:, mc, st * 128:(st + 1) * 128],
                                wv_["wv"][:, mc, :],
                                start=(mc == 0), stop=(mc == MC - 1))
                    va = pva.tile([128, ST * (DH + 1)], F16, name="va",
                                  tag="va")
                    nc.vector.memset(va[:], 1.0)
                    nc.vector.tensor_copy(
                        va[:].rearrange("p (s d) -> p s d", d=DH + 1)
                        [:, :, 0:DH],
                        ps_v[:].rearrange("p (s d) -> p s d", d=DH))
                    st_["va"] = va
                return run

            def qkv_fills(p):
                return [mk_qk_chunk(p, 0), mk_v(p), mk_qk_chunk(p, 1)]

            def mk_proj(p, st, z_lo, z_hi, obh_box):
                wo_t = wviews[grp[p]]["wo"]
                last = p == n_pairs - 1

                def run():
                    zh = z_lo if st < 4 else z_hi
                    zsl = zh[:, (st % 4) * 128:(st % 4 + 1) * 128]
                    gg = st % 4
                    if gg == 0:
                        obh_box[0] = pout.tile([128, 4, DM], F16,
                                               name="obh", tag="obh")
                    obh = obh_box[0]
                    pp = ppp.tile([128, 1024], F32, name="ps_p", tag="ps_p")
                    nc.tensor.matmul(pp[:, 512:898], zsl,
                                     wo_t[:, 384:DM + 2],
                                     start=True, stop=True)
                    rc = prc.tile([128, 1], F32, name=f"rc{st}",
                                  tag=f"rc{st}")
                    nc.vector.reciprocal(rc[:], pp[:, 896:897])
                    nc.tensor.matmul(pp[:, 0:384], zsl, wo_t[:, 0:384],
                                     start=True, stop=True)
                    in_ap = pp[:].rearrange(
                        "p (b c) -> p b c", c=512)[:, :, 0:384]
                    out_ap = obh[:, gg, :].rearrange(
                        "p (b c) -> p b c", c=384)
                    if st in ACT_STS:
                        nc.scalar.mul(out_ap, in_ap, rc[:])
                    else:
                        nc.vector.tensor_scalar(
                            out_ap, in_ap, rc[:], None,
                            op0=mybir.AluOpType.mult)
                    if last and gg % 2 == 1:
                        nc.gpsimd.dma_start(
                            out=out[p, (st - 1) * 128:(st + 1) * 128, :]
                            .rearrange("(g sp) m -> sp g m", sp=128),
                            in_=obh[:, gg - 1:gg + 1, :])
                    elif not last and gg == 3:
                        nc.gpsimd.dma_start(
                            out=out[p, (st - 3) * 128:(st + 1) * 128, :]
                            .rearrange("(g sp) m -> sp g m", sp=128),
                            in_=obh[:])
                return run

            def attention(p, z_lo, z_hi, fq_qkv, fq_proj, fq_late):
                st_ = state[p]
                ztile = [None]
                pend = {}

                def emit_pair(k):
                    j, a = SCHED[k]
                    LB = 256 if a < j else 128
                    c0 = 256 * j
                    c0B = c0 if a < j else c0 + 128
                    diag = a == j
                    kqA = st_[f"kq{a // 2}"]
                    kqB = st_[f"kq{(2 * a + 1) // 4}"]
                    qt = st_[f"qlo{j // 2}"]
                    qof = (j % 2) * 256
                    # one psum bank per block pair: only the FIRST matmul may
                    # use start=True (it clears has_written for the whole
                    # bank); the rest overwrite/accumulate per element.
                    T = ppr.tile([128, 512], F32, name="ps_s", tag="ps_r")
                    nc.tensor.matmul(
                        T[:, 0:256],
                        kqA[0:DH, (2 * a % 4) * 128:(2 * a % 4) * 128 + 128],
                        qt[:, qof:qof + 256], start=True, stop=False)
                    nc.tensor.matmul(
                        T[:, 256:256 + LB],
                        kqB[0:DH, ((2 * a + 1) % 4) * 128:
                            ((2 * a + 1) % 4 + 1) * 128],
                        qt[:, qof + c0B - c0:qof + c0B - c0 + LB],
                        start=False, stop=not diag)
                    if diag:
                        nc.tensor.matmul(T[:, 0:128], negI, ltm,
                                         start=False, stop=False)
                        nc.tensor.matmul(T[:, 256:384], negI, ltm,
                                         start=False, stop=True)
                    pend[k] = (T, LB)

                emit_pair(0)
                emit_pair(1)
                for k, (j, a) in enumerate(SCHED):
                    if a == 0 and j % 2 == 0:
                        ztile[0] = ppz.tile([DH + 1, 512], F32, name="ps_zb",
                                            tag="ps_z")
                    zt = ztile[0]
                    zoff = (j % 2) * 256
                    T, LB = pend.pop(k)
                    ex = pexp.tile([128, 512], F16, name="ex", tag="ex")
                    nc.scalar.activation(
                        ex[:, 0:256 + LB], T[:, 0:256 + LB],
                        mybir.ActivationFunctionType.Exp,
                        bias=0.0, scale=0.125)
                    if k + 2 < len(SCHED):
                        emit_pair(k + 2)
                    nc.tensor.matmul(
                        zt[:, zoff:zoff + 256],
                        st_["va"][:, 2 * a * (DH + 1):
                                  (2 * a + 1) * (DH + 1)],
                        ex[:, 0:256], start=(a == 0), stop=False)
                    nc.tensor.matmul(
                        zt[:, zoff + 256 - LB:zoff + 256],
                        st_["va"][:, (2 * a + 1) * (DH + 1):
                                  (2 * a + 2) * (DH + 1)],
                        ex[:, 256:256 + LB], start=False, stop=(a == j))
                    # fill slot: next-pair QKV at its slots, else previous-
                    # pair proj tiles, else (last pair, z_lo ready) own proj
                    # tiles, else a dummy warm matmul to keep the HAM gate
                    # at 8/8 through sparse sections.
                    for _rep in range(2 if k == len(SCHED) - 1 else 1):
                        if k in QKV_SLOTS and fq_qkv:
                            fq_qkv.pop(0)()
                        elif fq_proj:
                            fq_proj.pop(0)()
                        elif k >= 3 and fq_late:
                            fq_late.pop(0)()
                        else:
                            ps_w = ppp.tile([128, 1024], F32, name="ps_w2",
                                            tag="ps_p")
                            nc.tensor.matmul(ps_w[:, 0:512], wscr[:, 0:128],
                                             wscr[:], start=True, stop=True)
                    if a == j and j % 2 == 1:
                        if j == 1:
                            nc.scalar.copy(z_lo[:], zt[:])
                        else:
                            nc.vector.tensor_copy(z_hi[:], zt[:])

            # ---- prologue ----
            load_group(0)
            load_x(0, fine=True)
            for f in qkv_fills(0):
                f()
            load_x(1)

            fq_proj = []
            for p in range(n_pairs):
                if p + 2 < n_pairs:
                    load_x(p + 2)
                    if grp[p + 2] == 1 and 1 not in wviews:
                        load_group(1)
                fq_qkv = qkv_fills(p + 1) if p + 1 < n_pairs else []
                z_lo = pz.tile([DH + 1, 512], F16, name="z_lo", tag="z_lo")
                z_hi = pz.tile([DH + 1, 512], F16, name="z_hi", tag="z_hi")
                obh_box = [None]
                last = p == n_pairs - 1
                fq_late = ([mk_proj(p, st, z_lo, z_hi, obh_box)
                            for st in range(4)] if last else [])
                attention(p, z_lo, z_hi, fq_qkv, fq_proj, fq_late)
                assert not fq_qkv and not fq_proj
                rest = range(4, ST) if last else range(ST)
                fq_proj = fq_late + [mk_proj(p, st, z_lo, z_hi, obh_box)
                                     for st in rest]
            for f in fq_proj:
                f()

    nc.finalize()
    return nc


_NC_CACHE = {}


def _get_nc():
    if "nc" not in _NC_CACHE:
        _NC_CACHE["nc"] = _build_kernel()
    return _NC_CACHE["nc"]


def _core_pairs(c):
    """6 (batch, head) pairs for core c: head c x batches 0..3, plus half of
    head 8 + c//2 (2 batches)."""
    pairs = [(b, c) for b in range(B)]
    h2 = 8 + c // 2
    b0 = (c % 2) * 2
    pairs += [(b0, h2), (b0 + 1, h2)]
    return pairs


def _head_blob(W_Q, b_Q, W_K, b_K, W_V, b_V, W_O, b_O, h):
    wbh = np.zeros((128, WBL), np.float16)
    wqk = wbh[:, WQK0:WV0].reshape(128, MC, 128)
    wqk[:, :, 0:DH] = W_K[h].reshape(MC, 128, DH).transpose(1, 0, 2)
    wqk[:, :, DH:128] = W_Q[h].reshape(MC, 128, DH).transpose(1, 0, 2)
    wbh[:, WV0:WO0].reshape(128, MC, DH)[:] = \
        W_V[h].reshape(MC, 128, DH).transpose(1, 0, 2)
    wbh[0:DH, WO0:WO0 + DM] = W_O[h]
    wbh[DH, WO0:WO0 + DM] = b_V[h] @ W_O[h] + b_O / H
    wbh[DH, WO0 + DM] = 1.0
    bkq = np.concatenate([np.asarray(b_K[h], np.float32),
                          np.asarray(b_Q[h], np.float32)])
    wbh[:, BKQ0:BKQ0 + 2] = \
        np.ascontiguousarray(bkq).view(np.float16).reshape(128, 2)
    return wbh


def _make_core_inputs(x, W_Q, b_Q, W_K, b_K, W_V, b_V, W_O, b_O, c):
    pairs = _core_pairs(c)
    m = {
        "xT": np.empty((PAIRS_PER_CORE, DM, S), np.float16),
        "wb": np.empty((2, 128, WBL), np.float16),
    }
    for idx, (b, h) in enumerate(pairs):
        m["xT"][idx] = x[b, :, h, :].T
    args = (W_Q, b_Q, W_K, b_K, W_V, b_V, W_O, b_O)
    m["wb"][0] = _head_blob(*args, pairs[0][1])
    m["wb"][1] = _head_blob(*args, pairs[4][1])
    cm = np.zeros((128, 256), np.float16)
    cm[:, 0:128] = NMASK * np.eye(128, dtype=np.float16)
    ql = np.arange(128)
    cm[:, 128:256] = (ql[None, :] < ql[:, None]).astype(np.float16)
    m["cmask"] = cm
    return m


def kernel(normalized_resid_pre, W_Q, b_Q, W_K, b_K, W_V, b_V, W_O, b_O):
    x = np.ascontiguousarray(np.asarray(normalized_resid_pre, dtype=np.float32))
    args = tuple(np.asarray(a, dtype=np.float32)
                 for a in (W_Q, b_Q, W_K, b_K, W_V, b_V, W_O, b_O))

    nc = _get_nc()
    in_maps = [_make_core_inputs(x, *args, c) for c in range(N_CORES)]
    res = run_bass_kernel_spmd(nc, in_maps, list(range(N_CORES)))

    got = np.empty((B, S, H, DM), np.float32)
    for c in range(N_CORES):
        ro = np.asarray(res.results[c]["out"], np.float32)
        for idx, (b, h) in enumerate(_core_pairs(c)):
            got[b, :, h, :] = ro[idx]
    return got
